# revision 1
# baseline (speedup 1.0000x reference)
"""Trainium2 Bass kernel for nn_DecodeBox (YOLOv3-style box decode).

Contract: kernel(feat0, feat1, feat2) takes FULL inputs
  feat0 [32,255,19,19], feat1 [32,255,38,38], feat2 [32,255,76,76] (f32)
and returns the FULL output [32, 22743, 85] f32.

Strategy: pure data-parallel over batch (4 images per core, 8 cores).
Per core, per scale:
  - load per-(b,anchor) feature tiles with the 85 attrs on SBUF partitions
    ([85, HW], contiguous DRAM reads), sigmoid everything in place (ACT)
  - separately load compact [24, HW] xy/wh tiles (channels 0..3 of every
    (b, anchor) pair); fix them batched: sigmoid+grid via one
    scalar_tensor_tensor, exp with per-partition ln(anchor/608) bias
  - PE-transpose cells into PSUM, DVE-copy conf/cls columns to SBUF
    staging, stitch the 4 box columns from the transposed box tiles, DMA
    out. All cells go through E-blocks (E stride-E transposes per 128*E
    cells; E=4 bulk, E=2 for the 19x19 scale) so each SBUF partition
    holds E consecutive output rows -> >=680B DMA store elements (>=512B
    avoids HBM read-modify-write). The last block per (b,anchor) is
    shifted to end at H*W; overlap rows are double-written identically.
  - DMA issue is spread across all three paths (loads on GpSimd/SWDGE,
    stores round-robin over SP-HWDGE / ACT-HWDGE / SWDGE) so no single
    sequencer serializes on per-dma_start issue cost.
"""

import numpy as np

import concourse.bacc as bacc
import concourse.mybir as mybir
from concourse import masks, tile
from concourse.bass_utils import run_bass_kernel_spmd

F32 = mybir.dt.float32
AFT = mybir.ActivationFunctionType
ALU = mybir.AluOpType

N_CORES = 8
B_FULL = 32
B_LOCAL = B_FULL // N_CORES  # 4
ATTRS = 85
TOTAL_ROWS = 22743
SGQ = 2    # full E-blocks per staging tile / store (deep store pipeline)
GROUP = 6  # single chunks per PSUM group in the tail path

ANCHORS = np.array(
    [[10, 13], [16, 30], [33, 23], [30, 61], [62, 45], [59, 119],
     [116, 90], [156, 198], [373, 326]], dtype=np.float32)
MASKS_ = [[6, 7, 8], [3, 4, 5], [0, 1, 2]]
SCALES = [(19, 0), (38, 1083), (76, 5415)]  # (grid G, output row offset)


def _layout(hw: int):
    """-> (E, full_starts, shifted_start|None): the whole cell range is
    covered by E-blocks of 128*E cells (E=4 bulk, E=2 for the 19x19 scale);
    the last block is shifted to end at hw and overlap rows are
    double-written with identical values. No sub-512B store elements."""
    e = 4 if hw >= 512 else 2
    span = 128 * e
    nfull = hw // span
    fulls = [span * i for i in range(nfull)]
    shifted = hw - span if hw % span else None
    return e, fulls, shifted


def _groups(starts, n):
    return [starts[i:i + n] for i in range(0, len(starts), n)]


def _runs(grp, stride):
    runs, q = [], 0
    while q < len(grp):
        n = 1
        while q + n < len(grp) and grp[q + n] == grp[q] + stride * n:
            n += 1
        runs.append((q, n, grp[q]))
        q += n
    return runs


def host_consts():
    """grid{s} [24,HW] rows 0:12 = cellx/G, rows 12:24 = celly/G;
    bias{s} [24,1] row k*12 + b*3 + a = ln(anchor_dim_k/608)."""
    out = {}
    for s, (g, _off) in enumerate(SCALES):
        hw = g * g
        grid = np.empty((24, hw), np.float32)
        grid[0:12] = (np.arange(hw, dtype=np.float32) % g) / g
        grid[12:24] = (np.arange(hw, dtype=np.float32) // g) / g
        bias = np.zeros((24, 1), np.float32)
        for k in range(2):
            for j in range(12):
                a = j % 3
                bias[k * 12 + j, 0] = np.log(ANCHORS[MASKS_[s][a]][k] / 608.0)
        out[f"grid{s}"] = grid
        out[f"bias{s}"] = bias
    return out


def build_nc(repeat: int = 1):
    nc = bacc.Bacc("TRN2", target_bir_lowering=False, debug=False,
                   num_devices=N_CORES)
    feats, grids, biases = [], [], []
    for s, (g, _off) in enumerate(SCALES):
        feats.append(nc.dram_tensor(f"feat{s}", [B_LOCAL, 255, g, g], F32,
                                    kind="ExternalInput").ap())
        grids.append(nc.dram_tensor(f"grid{s}", [24, g * g], F32,
                                    kind="ExternalInput").ap())
        biases.append(nc.dram_tensor(f"bias{s}", [24, 1], F32,
                                     kind="ExternalInput").ap())
    out = nc.dram_tensor("out", [B_LOCAL, TOTAL_ROWS, ATTRS], F32,
                         kind="ExternalOutput").ap()

    with tile.TileContext(nc) as tc:
        with (
            tc.tile_pool(name="const", bufs=1) as const_pool,
            tc.tile_pool(name="box", bufs=1) as box_pool,
            tc.tile_pool(name="unit", bufs=3) as unit_pool,
            tc.tile_pool(name="bstage", bufs=17) as bstage_pool,
            tc.tile_pool(name="stage", bufs=12) as stage_pool,
            tc.tile_pool(name="pmain", bufs=6, space="PSUM") as psum_main,
            tc.tile_pool(name="pbox", bufs=2, space="PSUM") as psum_box,
        ):
            ident = const_pool.tile([128, 128], F32, tag="ident")
            masks.make_identity(nc, ident[:])

            grid_t, bias_t = {}, {}
            for s, (g, _off) in enumerate(SCALES):
                hw = g * g
                grid_t[s] = const_pool.tile([24, hw], F32, tag=f"grid{s}",
                                            name=f"grid_t{s}")
                nc.gpsimd.dma_start(grid_t[s][:], grids[s][:])
                bias_t[s] = const_pool.tile([24, 1], F32, tag=f"bias{s}",
                                            name=f"bias_t{s}")
                nc.gpsimd.dma_start(bias_t[s][:], biases[s][:])

            for _rep in range(repeat):
                _emit_pass(nc, tc, feats, out, grid_t, bias_t, ident,
                           box_pool, unit_pool, bstage_pool, stage_pool,
                           psum_main, psum_box)
    nc.compile()
    return nc


def _strided_cols(ap, start, espan, e):
    """ap[:, start + e :: espan] over espan*128 cells -> [P, 128] stride-espan
    column slice starting at cell start+e."""
    return ap[:, start:start + espan * 128] \
        .rearrange("p (f e) -> p e f", e=espan)[:, e, :]


def _emit_pass(nc, tc, feats, out, grid_t, bias_t, ident,
               box_pool, unit_pool, bstage_pool, stage_pool,
               psum_main, psum_box):
    xy_t, wh_t = {}, {}
    for s, (g, _off) in enumerate(SCALES):
        hw = g * g
        # box tiles: partition p = k*12 + b*3 + a (k in 0..1 each)
        src = feats[s].rearrange("b (a c) h w -> c b a (h w)", a=3)
        xy_t[s] = box_pool.tile([24, hw], F32, tag=f"boxxy{s}",
                                name=f"xy_t{s}")
        nc.gpsimd.dma_start(xy_t[s][:], src[0:2])
        wh_t[s] = box_pool.tile([24, hw], F32, tag=f"boxwh{s}",
                                name=f"wh_t{s}")
        nc.gpsimd.dma_start(wh_t[s][:], src[2:4])

    # cluster sigmoids together, then exps (ACT table loads cost 1.3us)
    for s in range(3):
        nc.scalar.activation(xy_t[s][:], xy_t[s][:], AFT.Sigmoid)
    for s in range(3):
        nc.scalar.activation(wh_t[s][:], wh_t[s][:], AFT.Exp,
                             bias=bias_t[s][:])
    for s, (g, _off) in enumerate(SCALES):
        # xy = sigmoid(p)/G + grid/G
        nc.vector.scalar_tensor_tensor(
            out=xy_t[s][:], in0=xy_t[s][:], scalar=1.0 / g,
            in1=grid_t[s][:], op0=ALU.mult, op1=ALU.add)

    store_rr = [nc.sync, nc.scalar, nc.sync, nc.scalar, nc.gpsimd]
    st_counter = [0]

    def _store(dst, src_ap):
        eng = store_rr[st_counter[0] % len(store_rr)]
        st_counter[0] += 1
        eng.dma_start(dst, src_ap)

    for s in (2, 1, 0):
        g, off = SCALES[s]
        hw = g * g
        e, fulls, shifted = _layout(hw)
        span = 128 * e
        blocks = fulls + ([shifted] if shifted is not None else [])

        # transpose the box tiles once per E-block; stage to SBUF (DMA can't
        # read PSUM and PSUM can't hold all blocks of a scale)
        bstages = []
        for st in blocks:
            pb = psum_box.tile([128, 48 * e], F32, tag="pbox")
            for ei in range(e):
                nc.tensor.transpose(pb[:, 48 * ei:48 * ei + 24],
                                    _strided_cols(xy_t[s][:], st, e, ei),
                                    ident[0:24, 0:24])
                nc.tensor.transpose(pb[:, 48 * ei + 24:48 * (ei + 1)],
                                    _strided_cols(wh_t[s][:], st, e, ei),
                                    ident[0:24, 0:24])
            bs = bstage_pool.tile([128, 48 * e], F32, tag="bstage")
            nc.vector.tensor_copy(bs[:], pb[:])
            bstages.append(bs)

        # units: s2 -> one [85,HW] tile per (b,a); s0/s1 -> one [85,3*HW]
        # tile per b covering all anchors
        if s == 2:
            units = [((b, (a,)), feats[s][b, 85 * a:85 * (a + 1)]
                      .rearrange("c h w -> c (h w)"), hw)
                     for b in range(B_LOCAL) for a in range(3)]
        else:
            units = [((b, (0, 1, 2)), feats[s][b]
                      .rearrange("(a c) h w -> c a (h w)", a=3), 3 * hw)
                     for b in range(B_LOCAL)]

        # store groups: SGQ consecutive full blocks share one staging tile
        # and one DMA; the shifted block stores separately (non-uniform row
        # stride vs the full blocks)
        sgroups = _groups(list(range(len(fulls))), SGQ)
        if shifted is not None:
            sgroups.append([len(fulls)])

        for (b, anchors), src_ap, ncols_u in units:
            ut = unit_pool.tile([85, ncols_u], F32, tag="unit")
            nc.gpsimd.dma_start(ut[:], src_ap)
            nc.scalar.activation(ut[:], ut[:], AFT.Sigmoid)
            for ai, a in enumerate(anchors):
                colbase = ai * hw
                j = b * 3 + a
                rbase = off + a * hw
                for sg in sgroups:
                    nq = len(sg)
                    stg = stage_pool.tile([128, ATTRS * e * nq], F32,
                                          tag="stage")
                    stgv = stg[:].rearrange("p (q e c) -> p q e c", e=e,
                                            c=ATTRS)
                    for qi, bi in enumerate(sg):
                        st = blocks[bi]
                        pm = psum_main.tile([128, ATTRS * e], F32,
                                            tag="pmain")
                        for ei in range(e):
                            nc.tensor.transpose(
                                pm[:, ATTRS * ei:ATTRS * (ei + 1)],
                                _strided_cols(ut[:], colbase + st, e, ei),
                                ident[0:85, 0:85])
                        pm3 = pm[:].rearrange("p (e c) -> p e c", c=ATTRS)
                        sl = stgv[:, qi, :, :]
                        nc.vector.tensor_copy(sl[:, :, 4:ATTRS],
                                              pm3[:, :, 4:ATTRS])
                        bsrc = bstages[bi][:].rearrange(
                            "p (e k j) -> p e k j", k=4, j=12)
                        nc.vector.tensor_copy(sl[:, :, 0:4],
                                              bsrc[:, :, :, j])
                    r0 = rbase + blocks[sg[0]]
                    dst = out[b, r0:r0 + nq * span, :] \
                        .rearrange("(q p e) c -> p q e c", p=128, e=e)
                    _store(dst, stgv[:, 0:nq, :, :])


_NC_CACHE = []


def _get_nc():
    if not _NC_CACHE:
        _NC_CACHE.append(build_nc())
    return _NC_CACHE[0]


def kernel(feat0, feat1, feat2):
    feats = [np.ascontiguousarray(np.asarray(f, dtype=np.float32))
             for f in (feat0, feat1, feat2)]
    assert feats[0].shape == (B_FULL, 255, 19, 19)
    assert feats[1].shape == (B_FULL, 255, 38, 38)
    assert feats[2].shape == (B_FULL, 255, 76, 76)

    consts = host_consts()
    nc = _get_nc()
    in_maps = []
    for c in range(N_CORES):
        m = dict(consts)
        for s in range(3):
            m[f"feat{s}"] = feats[s][c * B_LOCAL:(c + 1) * B_LOCAL]
        in_maps.append(m)

    res = run_bass_kernel_spmd(nc, in_maps, list(range(N_CORES)))
    return np.concatenate([res.results[c]["out"] for c in range(N_CORES)],
                          axis=0)

